# revision 1
# baseline (speedup 1.0000x reference)
"""Trainium2 Bass kernel for nn_Loss_1_8323646620405 (multi-head BCE/CCE loss).

Data-parallel over the batch dim: 8 cores x 8 batches each. Each core
computes per-partition partial sums of the (positive-log form) loss; the
host sums the 8x2x128xK partials, negates, and divides by B*S.

Self-contained: hardcodes shapes from the problem spec.
"""

import numpy as np

import concourse.bass as bass
import concourse.mybir as mybir
import concourse.tile as tile
from concourse.bass_utils import run_bass_kernel_spmd

# ---- walrus single-wait workaround ----------------------------------------
# This container's walrus build encodes at most ONE semaphore wait per
# instruction ('Too many sync wait commands'). Tile's scheduler freely
# attaches N waits to one instruction. Two patches:
#  1. postorder_instruction_blocks wrapper: split any instruction carrying
#     >1 wait -- extra waits move to same-engine NoOps inserted before it.
#  2. _drain_and_barrier: one drain per outstanding logical processor.
import bass_rust
from concourse.tile_cfg import postorder_instruction_blocks as _orig_post

_DMA_PROC_START = 10  # Collectives/DMASW*/DMAHW* procs inc by 16 per tick
_nop_ctr = [0]


def _split_waits_in_list(insts):
    out = []
    for ins in insts:
        si = getattr(ins, "sync_info", None)
        waits = list(si.on_wait) if si is not None else []
        if len(waits) > 1:
            for w in waits[:-1]:
                _nop_ctr[0] += 1
                nop = mybir.InstNoOp(name=f"WSPL-{_nop_ctr[0]}", ins=[], outs=[])
                nop.engine = ins.engine
                nop.sync_info = bass_rust.SyncInfo(on_wait=[w], on_update=[])
                out.append(nop)
            ins.sync_info = bass_rust.SyncInfo(
                on_wait=[waits[-1]], on_update=list(si.on_update)
            )
        out.append(ins)
    return out


def _patched_post(instructions, start_bb, output):
    for k in list(instructions.keys()):
        instructions[k] = _split_waits_in_list(instructions[k])
    return _orig_post(instructions, start_bb, output)


def _split_drain_and_barrier(self, tick_clock, wait_clock):
    gc = tick_clock.global_clock
    alloc = wait_clock.sems.allocated()
    for proc in sorted(alloc):
        tick = gc.peek_next(proc) - 1
        if tick <= 0:
            continue
        scale = 16 if proc >= _DMA_PROC_START else 1
        d = self.nc.sync.drain()
        d.wait_op(alloc[proc], tick * scale, "sem-ge")

    self.nc.all_engine_barrier()
    popped = self.nc._tile_sem_poison_stack.pop()
    assert popped is self._sem_poison
    self.nc.clear_and_free_semaphores(list(self.sems.allocated().values()))
    self.nc.all_engine_barrier()


tile.postorder_instruction_blocks = _patched_post
tile.TileContext._drain_and_barrier = _split_drain_and_barrier

# ---- problem constants -----------------------------------------------------
B, S, F = 64, 32768, 9
W0, W1 = 0.51, 19.05
C2 = W1 - W0
C1 = W0 + C2 / 2.0

NCORES = 8
B_LOC = B // NCORES          # 8 batches per core
N = B_LOC * S                # 262144 elements per core
P = 128                      # SBUF partitions
FD = N // P                  # 2048 free-dim elements per partition
CHUNK = 512                  # max free-dim elements per chunk
# head minis shrink the serial DMA->compute ramp of the first chunk
CHUNKS = [(0, 128), (128, 384), (512, 512), (1024, 512), (1536, 512)]
K = len(CHUNKS)

f32 = mybir.dt.float32
i32 = mybir.dt.int32
i16 = mybir.dt.int16
i8 = mybir.dt.int8
Alu = mybir.AluOpType
Act = mybir.ActivationFunctionType
X = mybir.AxisListType.X


def _build_nc() -> bass.Bass:
    nc = bass.Bass()

    # const AP for Ln bias=0.5 (same pattern as Bass.__init__ consts)
    c05 = nc.alloc_sbuf_tensor("const-float32-0.5", [P, 1], f32)
    nc.gpsimd.memset(c05.ap(), 0.5)
    nc.const_aps.aps[(f32, 0.5)] = c05.ap()
    nc.all_engine_barrier()

    ps_d = nc.declare_dram_parameter("y_pred_stroke", [N], f32, isOutput=False)
    pp_d = nc.declare_dram_parameter("y_pred_player", [N], f32, isOutput=False)
    ph_d = nc.declare_dram_parameter("y_pred_hand", [N], f32, isOutput=False)
    P3_d = nc.declare_dram_parameter("y_pred_point", [N * 3], f32, isOutput=False)
    Q4_d = nc.declare_dram_parameter("y_pred_serve", [N * 4], f32, isOutput=False)
    Y9_d = nc.declare_dram_parameter("y_target", [N * 9], i8, isOutput=False)
    acc_d = nc.declare_dram_parameter("acc", [P, 2 * K], f32, isOutput=True)


    with tile.TileContext(nc) as tc:
        with (
            tc.tile_pool(name="io", bufs=2) as io,
            tc.tile_pool(name="y9pool", bufs=2) as y9p,
            tc.tile_pool(name="tmp2", bufs=2) as tp2,
            tc.tile_pool(name="tmp1", bufs=1) as tp1,
            tc.tile_pool(name="acc", bufs=1) as ac,
        ):
            accT = ac.tile([P, 2 * K], f32)
            acc1 = accT[:, 0:K]
            acc2 = accT[:, K : 2 * K]

            y9_tiles = {}

            def load_y9(k):
                off, sz = CHUNKS[k]
                t = y9p.tile([P, sz, 9], i8, tag="Y9")
                v = Y9_d[9 * P * off : 9 * P * (off + sz)].rearrange(
                    "(p c d) -> p c d", p=P, d=9
                )
                nc.sync.dma_start(t[:], v)
                y9_tiles[k] = t

            # Y9 gates the longest compute chain (ACT int16 convert -> DVE
            # tree); issue Y9(k+1) right after chunk k's own inputs so the
            # convert overlaps chunk k's compute.
            load_y9(0)
            for k, (off, sz) in enumerate(CHUNKS):
                Y9 = y9_tiles.pop(k)
                ps_v = ps_d[P * off : P * (off + sz)].rearrange("(p c) -> p c", p=P)
                pp_v = pp_d[P * off : P * (off + sz)].rearrange("(p c) -> p c", p=P)
                ph_v = ph_d[P * off : P * (off + sz)].rearrange("(p c) -> p c", p=P)
                P3_v = P3_d[3 * P * off : 3 * P * (off + sz)].rearrange("(p c d) -> p c d", p=P, d=3)
                Q4_v = Q4_d[4 * P * off : 4 * P * (off + sz)].rearrange("(p c d) -> p c d", p=P, d=4)
                ps = io.tile([P, sz], f32, tag="ps")
                pp = io.tile([P, sz], f32, tag="pp")
                ph = io.tile([P, sz], f32, tag="ph")
                P3 = io.tile([P, sz, 3], f32, tag="P3")
                Q4 = io.tile([P, sz, 4], f32, tag="Q4")
                nc.sync.dma_start(ps[:], ps_v)
                nc.sync.dma_start(P3[:], P3_v)
                nc.sync.dma_start(Q4[:], Q4_v)
                nc.sync.dma_start(pp[:], pp_v)
                nc.sync.dma_start(ph[:], ph_v)
                if k + 1 < K:
                    load_y9(k + 1)

                Y16 = tp2.tile([P, sz * 9], i16, tag="Y16")
                A = tp1.tile([P, sz // 2, 4], i32, tag="A")
                Bt = tp1.tile([P, sz // 2, 2], i32, tag="Bt")
                Cs = tp1.tile([P, sz // 2], i32, tag="Cs")
                S32 = tp1.tile([P, sz // 2], i32, tag="S32")
                u = tp1.tile([P, sz], f32, tag="u")
                u0 = tp2.tile([P, sz], f32, tag="u0")
                u7 = tp2.tile([P, sz], f32, tag="u7")
                m_s = tp1.tile([P, sz], f32, tag="m_s")
                L_s = tp2.tile([P, sz], f32, tag="L_s")
                m_p = tp1.tile([P, sz], f32, tag="m_p")
                ppe = tp2.tile([P, sz], f32, tag="ppe")
                m_h = tp1.tile([P, sz], f32, tag="m_h")
                phe = tp2.tile([P, sz], f32, tag="phe")
                d5 = tp1.tile([P, sz], f32, tag="d5")
                t5 = tp1.tile([P, sz], f32, tag="t5")
                i5 = tp1.tile([P, sz], f32, tag="i5")
                d4 = tp1.tile([P, sz], f32, tag="d4")
                t4 = tp1.tile([P, sz], f32, tag="t4")
                d6 = tp1.tile([P, sz], f32, tag="d6")
                t6 = tp1.tile([P, sz], f32, tag="t6")
                i6 = tp1.tile([P, sz], f32, tag="i6")
                d3 = tp1.tile([P, sz], f32, tag="d3")
                t3 = tp1.tile([P, sz], f32, tag="t3")
                i3 = tp1.tile([P, sz], f32, tag="i3")
                d2 = tp1.tile([P, sz], f32, tag="d2")
                t2 = tp1.tile([P, sz], f32, tag="t2")
                psel_pt = tp1.tile([P, sz], f32, tag="psel_pt")
                psel_sv = tp1.tile([P, sz], f32, tag="psel_sv")
                pr1 = tp1.tile([P, sz], f32, tag="pr1")
                pr2 = tp1.tile([P, sz], f32, tag="pr2")
                Pi = tp1.tile([P, sz], f32, tag="Pi")
                L_Pi = tp2.tile([P, sz], f32, tag="L_Pi")
                R = tp1.tile([P, sz], f32, tag="R")
                dum2 = tp1.tile([P, sz], f32, tag="dum2")

                # --- s = any(y==1): int16 pack (interleaved pairs) + int32 lane adds.
                # ACT converts y to int16 with slot = c2*18 + d*2 + par, so each
                # int32 word holds (elem 2c2, elem 2c2+1) lane-separated; summing
                # 9 words gives both elements' sum9 with no cross-lane carries.
                Y16v = Y16[:].rearrange("p (c2 d par) -> p c2 par d", d=9, par=2)
                Y9v = Y9[:].rearrange("p (c2 par) d -> p c2 par d", par=2)
                nc.scalar.activation(Y16v, Y9v, Act.Copy)
                X9 = Y16[:].bitcast(i32).rearrange("p (c d) -> p c d", d=9)
                nc.vector.tensor_tensor(A[:], X9[:, :, 0:4], X9[:, :, 4:8], op=Alu.add)
                nc.vector.tensor_tensor(Bt[:], A[:, :, 0:2], A[:, :, 2:4], op=Alu.add)
                nc.vector.tensor_tensor(Cs[:], Bt[:, :, 0], Bt[:, :, 1], op=Alu.add)
                nc.vector.tensor_tensor(S32[:], Cs[:], X9[:, :, 8], op=Alu.add)
                S16 = S32[:].bitcast(i16)  # [P, sz] int16 sum9, element order
                # u = (sum9 >= 1) - 0.5 in {-.5, +.5}
                nc.vector.tensor_scalar(u[:], S16, 0.5, 0.5, Alu.is_ge, Alu.subtract)
                # ACT casts int32 -> f32 (strided reads)
                nc.scalar.activation(u0[:], Y9[:, :, 0], Act.Copy, bias=-0.5, scale=1.0)
                nc.scalar.activation(u7[:], Y9[:, :, 7], Act.Copy, bias=-0.5, scale=1.0)
                # stroke: L_s = ln(p_eff) with p_eff = 0.5 + u*(2ps-1)
                nc.vector.scalar_tensor_tensor(m_s[:], ps[:], 0.5, u[:], Alu.subtract, Alu.mult)
                # acc1[:,k] = sum(L_s); host multiplies by W0. The s-dependent
                # (W1-W0)*s*L_s part rides the term2 chain below.
                nc.scalar.activation(L_s[:], m_s[:], Act.Ln, bias=0.5, scale=2.0,
                                     accum_out=acc1[:, k : k + 1])
                # player/hand effective probs: 0.5 - 2*(p-0.5)*(y-0.5)
                nc.vector.scalar_tensor_tensor(m_p[:], pp[:], 0.5, u0[:], Alu.subtract, Alu.mult)
                nc.scalar.activation(ppe[:], m_p[:], Act.Copy, bias=0.5, scale=-2.0)
                nc.vector.scalar_tensor_tensor(m_h[:], ph[:], 0.5, u7[:], Alu.subtract, Alu.mult)
                nc.scalar.activation(phe[:], m_h[:], Act.Copy, bias=0.5, scale=-2.0)
                # point select: y4 ? P0 : (y5 ? P1 : P2); int32 masks read directly
                nc.vector.tensor_sub(d5[:], P3[:, :, 1], P3[:, :, 2])
                nc.vector.tensor_mul(t5[:], Y9[:, :, 5], d5[:])
                nc.vector.tensor_add(i5[:], t5[:], P3[:, :, 2])
                nc.vector.tensor_sub(d4[:], P3[:, :, 0], i5[:])
                nc.vector.tensor_mul(t4[:], Y9[:, :, 4], d4[:])
                nc.vector.tensor_add(psel_pt[:], t4[:], i5[:])
                # serve select: y2 ? Q0 : (y3 ? Q1 : (y6 ? Q2 : Q3))
                nc.vector.tensor_sub(d6[:], Q4[:, :, 2], Q4[:, :, 3])
                nc.vector.tensor_mul(t6[:], Y9[:, :, 6], d6[:])
                nc.vector.tensor_add(i6[:], t6[:], Q4[:, :, 3])
                nc.vector.tensor_sub(d3[:], Q4[:, :, 1], i6[:])
                nc.vector.tensor_mul(t3[:], Y9[:, :, 3], d3[:])
                nc.vector.tensor_add(i3[:], t3[:], i6[:])
                nc.vector.tensor_sub(d2[:], Q4[:, :, 0], i3[:])
                nc.vector.tensor_mul(t2[:], Y9[:, :, 2], d2[:])
                nc.vector.tensor_add(psel_sv[:], t2[:], i3[:])
                # Pi = ppe*phe*psel_pt*psel_sv ; L_Pi = ln(Pi)
                nc.vector.tensor_mul(pr1[:], ppe[:], phe[:])
                nc.vector.tensor_mul(pr2[:], psel_pt[:], psel_sv[:])
                nc.vector.tensor_mul(Pi[:], pr1[:], pr2[:])
                nc.scalar.activation(L_Pi[:], Pi[:], Act.Ln)
                # R = (W1-W0)*L_s + L_Pi ; acc2[:,k] = sum(s * R)
                nc.vector.scalar_tensor_tensor(R[:], L_s[:], C2, L_Pi[:], Alu.mult, Alu.add)
                nc.vector.scalar_tensor_tensor(
                    dum2[:], u[:], 0.5, R[:], Alu.add, Alu.mult,
                    accum_out=acc2[:, k : k + 1],
                )

            nc.sync.dma_start(acc_d[:], accT[:])

    return nc


_NC_CACHE = None


def _get_nc():
    global _NC_CACHE
    if _NC_CACHE is None:
        _NC_CACHE = _build_nc()
    return _NC_CACHE


def _shard_inputs(inputs):
    in_maps = []
    for i in range(NCORES):
        sl = slice(i * B_LOC, (i + 1) * B_LOC)
        in_maps.append(
            {
                "y_pred_stroke": np.ascontiguousarray(
                    inputs["y_pred_stroke"][sl], dtype=np.float32
                ).reshape(-1),
                "y_pred_player": np.ascontiguousarray(
                    inputs["y_pred_player"][sl], dtype=np.float32
                ).reshape(-1),
                "y_pred_hand": np.ascontiguousarray(
                    inputs["y_pred_hand"][sl], dtype=np.float32
                ).reshape(-1),
                "y_pred_point": np.ascontiguousarray(
                    inputs["y_pred_point"][sl], dtype=np.float32
                ).reshape(-1),
                "y_pred_serve": np.ascontiguousarray(
                    inputs["y_pred_serve"][sl], dtype=np.float32
                ).reshape(-1),
                # lossless 0/1 cast: 4x less HBM traffic for the target tensor
                "y_target": np.ascontiguousarray(
                    inputs["y_target"][sl], dtype=np.int8
                ).reshape(-1),
            }
        )
    return in_maps


def kernel(**inputs) -> np.ndarray:
    nc = _get_nc()
    in_maps = _shard_inputs(inputs)
    res = run_bass_kernel_spmd(nc, in_maps, list(range(NCORES)))
    total = 0.0
    for r in res.results:
        a = r["acc"].astype(np.float64)
        total += W0 * a[:, :K].sum() + a[:, K:].sum()
    mean = -total / float(B * S)
    return np.array([mean], dtype=np.float32)



# revision 23
# speedup vs baseline: 2.0685x; 2.0685x over previous
"""Trainium2 Bass kernel for nn_Loss_1_8323646620405 (multi-head BCE/CCE loss).

Data-parallel over batch: 8 cores x 8 batches. Host re-encodes inputs into a
single plane-major uint16 array per core (packbits labels + bf16 pred planes,
binary-head planes pre-shifted by -0.5) so each chunk is ONE DMA. Device:
  s    = any(y)        -> u = s-0.5            (DVE tensor_scalar on int16)
  L_s  = ln(0.5+2(ps-.5)u)                      (ACT Ln, accum -> acc1)
  ppe  = 0.5-2(pp-.5)(y0-.5), phe likewise      (DVE TT + ACT affine copy)
  Psel = P3[point idx], Qsel = Q4[serve idx]    (copy_predicated cascades, Pool)
  Pi   = ppe*phe*Psel*Qsel ; L_Pi = ln(Pi)      (DVE TT, ACT Ln)
  acc2 = sum s*(C2*L_s + L_Pi)                  (DVE STT accum)
loss = -(W0*acc1 + acc2)/(B*S) summed on host.
"""

import numpy as np
import ml_dtypes

import concourse.bass as bass
import concourse.mybir as mybir
import concourse.tile as tile
from concourse.bass_utils import run_bass_kernel_spmd

# ---- walrus single-wait workaround ----------------------------------------
# This container's walrus build encodes at most ONE semaphore wait per
# instruction ('Too many sync wait commands'). Tile's scheduler freely
# attaches N waits to one instruction. Two patches:
#  1. postorder_instruction_blocks wrapper: split any instruction carrying
#     >1 wait -- extra waits move to same-engine NoOps inserted before it.
#  2. _drain_and_barrier: one drain per outstanding logical processor.
import bass_rust
from concourse.tile_cfg import postorder_instruction_blocks as _orig_post

_DMA_PROC_START = 10  # Collectives/DMASW*/DMAHW* procs inc by 16 per tick
_nop_ctr = [0]


def _split_waits_in_list(insts):
    out = []
    for ins in insts:
        si = getattr(ins, "sync_info", None)
        waits = list(si.on_wait) if si is not None else []
        if len(waits) > 1:
            for w in waits[:-1]:
                _nop_ctr[0] += 1
                nop = mybir.InstNoOp(name=f"WSPL-{_nop_ctr[0]}", ins=[], outs=[])
                nop.engine = ins.engine
                nop.sync_info = bass_rust.SyncInfo(on_wait=[w], on_update=[])
                out.append(nop)
            ins.sync_info = bass_rust.SyncInfo(
                on_wait=[waits[-1]], on_update=list(si.on_update)
            )
        out.append(ins)
    return out


def _patched_post(instructions, start_bb, output):
    for k in list(instructions.keys()):
        instructions[k] = _split_waits_in_list(instructions[k])
    return _orig_post(instructions, start_bb, output)


def _split_drain_and_barrier(self, tick_clock, wait_clock):
    gc = tick_clock.global_clock
    alloc = wait_clock.sems.allocated()
    for proc in sorted(alloc):
        tick = gc.peek_next(proc) - 1
        if tick <= 0:
            continue
        scale = 16 if proc >= _DMA_PROC_START else 1
        d = self.nc.sync.drain()
        d.wait_op(alloc[proc], tick * scale, "sem-ge")

    self.nc.all_engine_barrier()
    popped = self.nc._tile_sem_poison_stack.pop()
    assert popped is self._sem_poison
    self.nc.clear_and_free_semaphores(list(self.sems.allocated().values()))
    self.nc.all_engine_barrier()


tile.postorder_instruction_blocks = _patched_post
tile.TileContext._drain_and_barrier = _split_drain_and_barrier

# ---- problem constants -----------------------------------------------------
B, S, F = 64, 32768, 9
W0, W1 = 0.51, 19.05
C2 = W1 - W0

NCORES = 8
B_LOC = B // NCORES          # 8 batches per core
N = B_LOC * S                # 262144 elements per core
P = 128                      # SBUF partitions
FD = N // P                  # 2048 free-dim elements per partition
NPLANES = 11                 # 0: labels, 1-10: bf16 pred planes
# small head chunk primes the DMA->compute pipeline
CHUNKS = [(0, 128), (128, 512), (640, 704), (1344, 704)]
K = len(CHUNKS)

f32 = mybir.dt.float32
bf16 = mybir.dt.bfloat16
i16 = mybir.dt.int16
u16 = mybir.dt.uint16
Alu = mybir.AluOpType
Act = mybir.ActivationFunctionType


def _build_nc() -> bass.Bass:
    nc = bass.Bass()

    # const AP for Ln bias=0.5 (same pattern as Bass.__init__ consts)
    c05 = nc.alloc_sbuf_tensor("const-float32-0.5", [P, 1], f32)
    nc.gpsimd.memset(c05.ap(), 0.5)
    nc.const_aps.aps[(f32, 0.5)] = c05.ap()
    nc.all_engine_barrier()

    D_d = nc.declare_dram_parameter("D", [NPLANES * N], bf16, isOutput=False)
    acc_d = nc.declare_dram_parameter("acc", [P, 4 * K], f32, isOutput=True)

    Dv = D_d.rearrange("(k p c) -> p k c", k=NPLANES, p=P)

    with tile.TileContext(nc) as tc:
        with (
            tc.tile_pool(name="io", bufs=2) as io,
            tc.tile_pool(name="tmp", bufs=2) as tp,
            tc.tile_pool(name="acc", bufs=1) as ac,
        ):
            accT = ac.tile([P, 4 * K], f32)
            acc1 = accT[:, 0:K]       # sum L_s            (ACT accum)
            acc2a = accT[:, K : 2 * K]    # sum u*L_s      (DVE accum)
            acc2b = accT[:, 2 * K : 3 * K]  # sum u*L_Pi   (DVE accum)
            acc3 = accT[:, 3 * K : 4 * K]   # sum L_Pi     (ACT accum)

            tiles = {}

            def load(k):
                off, sz = CHUNKS[k]
                T = io.tile([P, NPLANES, sz], bf16, tag="T")
                nc.sync.dma_start(T[:], Dv[:, :, off : off + sz])
                tiles[k] = T

            load(0)
            for k, (off, sz) in enumerate(CHUNKS):
                T = tiles.pop(k)
                if k + 1 < K:
                    load(k + 1)
                yb = T[:, 0, :].bitcast(u16)
                psh = T[:, 1, :]   # ps - 0.5 (host pre-shifted)
                pph = T[:, 2, :]   # pp - 0.5
                phh = T[:, 3, :]   # ph - 0.5
                P0 = T[:, 4, :]
                P1 = T[:, 5, :]
                Psel = T[:, 6, :]  # P2 plane, overwritten in place by selects
                Q0 = T[:, 7, :]
                Q1 = T[:, 8, :]
                Q2 = T[:, 9, :]
                Qsel = T[:, 10, :]  # Q3 plane, overwritten in place

                u = tp.tile([P, sz], bf16, tag="u")
                v0 = tp.tile([P, sz], bf16, tag="v0")
                u7m = tp.tile([P, sz], u16, tag="u7m")
                v7 = tp.tile([P, sz], bf16, tag="v7")
                y5m = tp.tile([P, sz], u16, tag="y5m")
                y4m = tp.tile([P, sz], u16, tag="y4m")
                y6m = tp.tile([P, sz], u16, tag="y6m")
                y3m = tp.tile([P, sz], u16, tag="y3m")
                y2m = tp.tile([P, sz], u16, tag="y2m")
                m_s = tp.tile([P, sz], bf16, tag="m_s")
                Xp = tp.tile([P, sz], bf16, tag="Xp")
                Xh = tp.tile([P, sz], bf16, tag="Xh")
                L_s = tp.tile([P, sz], bf16, tag="L_s")
                ppe = tp.tile([P, sz], bf16, tag="ppe")
                phe = tp.tile([P, sz], bf16, tag="phe")
                pr1 = tp.tile([P, sz], bf16, tag="pr1")
                pr2 = tp.tile([P, sz], bf16, tag="pr2")
                Pi = tp.tile([P, sz], bf16, tag="Pi")
                L_Pi = tp.tile([P, sz], bf16, tag="L_Pi")
                sLs = tp.tile([P, sz], bf16, tag="sLs")
                sR = tp.tile([P, sz], bf16, tag="sR")
                dq1 = tp.tile([P, sz], bf16, tag="dq1")
                dq2 = tp.tile([P, sz], bf16, tag="dq2")

                # label-derived masks. yb is uint16 with host bit layout:
                # bit15=y0, bit14=y7, bits0-4 = y2,y3,y4,y5,y6. Unsigned
                # compares extract the high bits in one tensor_scalar each
                # (walrus requires op0/op1 to share the bitwise/arith class).
                nc.vector.tensor_scalar(u[:], yb, 0, 0.5, Alu.is_gt, Alu.subtract)
                nc.vector.tensor_scalar(v0[:], yb, 32767, 0.5, Alu.is_gt, Alu.subtract)
                nc.vector.tensor_scalar(u7m[:], yb, 16384, None, Alu.bitwise_and)
                nc.vector.tensor_scalar(v7[:], u7m[:], 8192.0, None, Alu.subtract)
                nc.vector.tensor_scalar(y5m[:], yb, 8, None, Alu.bitwise_and)
                nc.vector.tensor_scalar(y4m[:], yb, 4, None, Alu.bitwise_and)
                nc.vector.tensor_scalar(y6m[:], yb, 16, None, Alu.bitwise_and)
                nc.vector.tensor_scalar(y3m[:], yb, 2, None, Alu.bitwise_and)
                nc.vector.tensor_scalar(y2m[:], yb, 1, None, Alu.bitwise_and)

                # stroke: L_s = ln(0.5 + 2*(ps-0.5)*u), accumulate per chunk
                nc.vector.tensor_tensor(m_s[:], psh, u[:], op=Alu.mult)
                nc.scalar.activation(L_s[:], m_s[:], Act.Ln, bias=0.5, scale=2.0,
                                     accum_out=acc1[:, k : k + 1])
                # player: v0 = y0-0.5, Xp = (pp-.5)(y0-.5), ppe = 0.5-2*Xp
                nc.vector.tensor_tensor(Xp[:], pph, v0[:], op=Alu.mult)
                nc.scalar.activation(ppe[:], Xp[:], Act.Copy, bias=0.5, scale=-2.0)
                # hand: v7 = 16384*(y7-0.5), phe = 0.5 - Xh/8192
                nc.gpsimd.tensor_tensor(Xh[:], phh, v7[:], op=Alu.mult)
                nc.scalar.activation(phe[:], Xh[:], Act.Copy, bias=0.5, scale=-1.0 / 8192.0)
                # point select: y4 ? P0 : (y5 ? P1 : P2)  (in place on P2 plane;
                # copy_predicated is DVE-only on Trn2)
                nc.vector.copy_predicated(Psel, y5m[:], P1)
                nc.vector.copy_predicated(Psel, y4m[:], P0)
                # serve select: y2 ? Q0 : (y3 ? Q1 : (y6 ? Q2 : Q3))
                nc.vector.copy_predicated(Qsel, y6m[:], Q2)
                nc.vector.copy_predicated(Qsel, y3m[:], Q1)
                nc.vector.copy_predicated(Qsel, y2m[:], Q0)
                # Pi = ppe*phe*Psel*Qsel ; L_Pi = ln(Pi)
                nc.gpsimd.tensor_tensor(pr1[:], ppe[:], phe[:], op=Alu.mult)
                nc.vector.tensor_tensor(pr2[:], Psel, Qsel, op=Alu.mult)
                nc.gpsimd.tensor_tensor(Pi[:], pr1[:], pr2[:], op=Alu.mult)
                nc.scalar.activation(L_Pi[:], Pi[:], Act.Ln,
                                     accum_out=acc3[:, k : k + 1])
                # masked accums via u=s-0.5: sum s*X = sum u*X + sum X / 2
                # (host adds the acc1/acc3 halves). TT on Pool, accum on DVE.
                nc.gpsimd.tensor_tensor(sLs[:], u[:], L_s[:], op=Alu.mult)
                nc.vector.tensor_scalar(
                    dq1[:], sLs[:], 0.0, None, Alu.add, Alu.add,
                    accum_out=acc2a[:, k : k + 1],
                )
                nc.gpsimd.tensor_tensor(sR[:], u[:], L_Pi[:], op=Alu.mult)
                nc.vector.tensor_scalar(
                    dq2[:], sR[:], 0.0, None, Alu.add, Alu.add,
                    accum_out=acc2b[:, k : k + 1],
                )

            nc.sync.dma_start(acc_d[:], accT[:])

    return nc


_NC_CACHE = None


def _get_nc():
    global _NC_CACHE
    if _NC_CACHE is None:
        _NC_CACHE = _build_nc()
    return _NC_CACHE


def _shard_inputs(inputs):
    bf = ml_dtypes.bfloat16
    # bf16 planes (computed once on full arrays, then sliced per core)
    planes_full = [
        (inputs["y_pred_stroke"][..., 0] - 0.5).astype(bf).view(np.uint16),
        (inputs["y_pred_player"][..., 0] - 0.5).astype(bf).view(np.uint16),
        (inputs["y_pred_hand"][..., 0] - 0.5).astype(bf).view(np.uint16),
    ] + [
        inputs["y_pred_point"][..., j].astype(bf).view(np.uint16) for j in range(3)
    ] + [
        inputs["y_pred_serve"][..., j].astype(bf).view(np.uint16) for j in range(4)
    ]
    # labels -> uint16 with layout: bits0-4 = y2,y3,y4,y5,y6; bit5=y1;
    # bit6=y8; bit14=y7; bit15=y0 (high bits enable unsigned-compare tricks)
    y = inputs["y_target"].astype(np.uint8)
    cols = np.zeros(y.shape[:-1] + (16,), np.uint8)
    for bit, j in [(0, 2), (1, 3), (2, 4), (3, 5), (4, 6), (5, 1), (6, 8), (14, 7), (15, 0)]:
        cols[..., bit] = y[..., j]
    yb_full = (
        np.packbits(cols.reshape(-1, 16), axis=-1, bitorder="little")
        .view(np.uint16)
        .reshape(B, S)
    )
    in_maps = []
    for i in range(NCORES):
        sl = slice(i * B_LOC, (i + 1) * B_LOC)
        D = np.empty((NPLANES, N), np.uint16)
        D[0] = yb_full[sl].reshape(-1)
        for j, pf in enumerate(planes_full):
            D[1 + j] = pf[sl].reshape(-1)
        in_maps.append({"D": D.reshape(-1).view(ml_dtypes.bfloat16)})
    return in_maps


def kernel(**inputs) -> np.ndarray:
    nc = _get_nc()
    in_maps = _shard_inputs(inputs)
    res = run_bass_kernel_spmd(nc, in_maps, list(range(NCORES)))
    total = 0.0
    for r in res.results:
        a = r["acc"].astype(np.float64)
        a1 = a[:, :K].sum()            # sum L_s
        a2a = a[:, K : 2 * K].sum()    # sum u*L_s
        a2b = a[:, 2 * K : 3 * K].sum()  # sum u*L_Pi
        a3 = a[:, 3 * K : 4 * K].sum()   # sum L_Pi
        # sum s*X = sum u*X + 0.5*sum X
        total += W0 * a1 + C2 * (a2a + 0.5 * a1) + (a2b + 0.5 * a3)
    mean = -total / float(B * S)
    return np.array([mean], dtype=np.float32)
